# revision 1
# baseline (speedup 1.0000x reference)
"""Trainium2 Bass/Tile kernel for the AttentionModule problem — fp8 version.

Computation (per the reference):
    h_proj  = hidden @ Wa[:, :D].T + ba                       [B, 2E]   (host)
    e_proj  = einsum('tbe,fe->tbf', enc, Wa[:, D:])           [T, B, 2E] (PE, fp8 DoubleRow)
    act     = tanh(h_proj + e_proj)                           (ACT, bias=h_projT)
    scores  = einsum('tbf,f->bt', act, w2[0])                 (PE, fp8 DoubleRow pairs)
    weights = softmax(scores, axis=t)                         (DVE/ACT)
    applied = einsum('bt,tbe->be', weights, enc)              (DVE stt, bf16)
    out     = tanh(cat(decoder_out, applied) @ Wc.T + bc)     (decoder half on host;
                                                               applied half on PE, fp8,
                                                               WcE x16 / psum x1/256)

Strategy: data-parallel over batch B=64 across 8 NeuronCores (8 rows each).
The dominant e_proj matmul (8.6 GMAC/core) runs in fp8e4m3 with
perf_mode=DoubleRow (2 fp8 weights/cell -> 2x bf16 peak).  Everything that
depends only on inputs (h_proj, decoder_out @ Wc[:, :D].T + bc) is folded on
the host; `applied` is unsharded/transposed on the host from the per-core
[e_tile, 128, b] accumulator layout.

The whole PE stream is fp8 (e_proj + scores DoubleRow, combine plain fp8) —
mixing bf16 matmuls into the fp8 DoubleRow stream measured ~40% slower
(PE mode switching).  The `applied` reduction stays bf16 on the DVE so that
output keeps its precision.  fp8e4m3 normals start at 2^-6, so small weights
are pre-scaled up on host: w2 x64 (undone via Exp scale=1/64), WcE x16 with
applied x16 (undone by 1/256 in the decoder-add before the final tanh).

Tile pools are created once and shared across reps (the timing harness chains
reps in one NEFF): buffer rotation then lets rep r+1's input DMAs overlap rep
r's softmax/combine tail instead of serializing at the rep boundary.  Score
matmuls are emitted one pair-slot late (crossing batch-row boundaries) so the
in-order PE queue always has independent e_proj work ahead of any tanh
dependency.
"""

import numpy as np
import ml_dtypes
from contextlib import ExitStack

import concourse.bass as bass
import concourse.tile as tile
from concourse import bacc, mybir
from concourse.bass_utils import run_bass_kernel_spmd

B, T, E, D = 64, 512, 1024, 1024
NCORES = 8
BL = B // NCORES          # 8 batch rows per core
F = 2 * E                 # 2048
KO = E // 128             # 8 contraction sub-tiles for e
KP = KO // 2              # 4 DoubleRow k-pairs
FJ = F // 128             # 16 f-tiles
JP = FJ // 2              # 8 f-tile pairs
F8 = mybir.dt.float8e4
BF16 = mybir.dt.bfloat16
F32 = mybir.dt.float32
AF = mybir.ActivationFunctionType
ALU = mybir.AluOpType
PM = mybir.MatmulPerfMode.DoubleRow

_nc_cache = None


def _bcast(row_ap, n=128):
    """[1, X] AP -> [n, X] partition-broadcast read AP (stride-0 partitions)."""
    return bass.AP(
        tensor=row_ap.tensor, offset=row_ap.offset,
        ap=[[0, n]] + [list(p) for p in row_ap.ap[1:]])


def _rep(tc, P, ins, wscr, out_d, appT_d, uid=""):
    nc = tc.nc

    # ---- constant / input loads (issue order = need order) ----
    waET = P["waET"].tile([128, KO, F], F8, name=f"waET{uid}", tag="waET")
    nc.sync.dma_start(out=waET, in_=ins["waET8"])
    enc = []
    for b in range(2):
        t_e = P["enc"].tile([128, KO, T], F8, name=f"enc{b}{uid}", tag=f"enc{b}")
        nc.sync.dma_start(out=t_e, in_=ins["enc8"][b])
        enc.append(t_e)
    hpT = P["w"].tile([128, FJ, BL], F32, name=f"hpT{uid}", tag="hpT")
    nc.sync.dma_start(out=hpT, in_=ins["hpT"])
    w2s = P["w"].tile([128, FJ, 16], F8, name=f"w2s{uid}", tag="w2s")
    nc.sync.dma_start(out=w2s, in_=ins["w2s"])
    for b in range(2, BL):
        t_e = P["enc"].tile([128, KO, T], F8, name=f"enc{b}{uid}", tag=f"enc{b}")
        nc.sync.dma_start(out=t_e, in_=ins["enc8"][b])
        enc.append(t_e)
    encb = []
    for b in range(BL):
        t_b = P["encb"].tile([128, KO, T], BF16, name=f"encb{b}{uid}",
                             tag=f"encb{b}")
        nc.sync.dma_start(out=t_b, in_=ins["encb"][b])
        encb.append(t_b)
    wces = P["tailc"].tile([128, KO, D], F8, name=f"wces{uid}", tag="wces")
    nc.sync.dma_start(out=wces, in_=ins["wces8"])
    dec = P["tailc"].tile([BL, D], F32, name=f"dec{uid}", tag="dec")
    nc.sync.dma_start(out=dec, in_=ins["dec"])

    # applied^T accumulators [e_tile][128, b] (fp32, written once per column)
    appT = []
    for k in range(KO):
        appT.append(P["work"].tile([128, BL], F32, name=f"appT{k}{uid}",
                                   tag=f"appT{k}"))

    # ---- per-b softmax + applied emission helper ----
    def _tail_b(b, ps):
        # softmax over t on one partition (psum holds 64*scores)
        negmax = P["sm"].tile([1, 1], F32, name=f"negmax{b}{uid}", tag="negmax")
        nc.vector.reduce_max(negmax, ps, axis=mybir.AxisListType.X, negate=True)
        nm64 = P["sm"].tile([1, 1], F32, name=f"nm64{b}{uid}", tag="nm64")
        nc.vector.tensor_scalar_mul(nm64, negmax, 1.0 / 64.0)
        wexp = P["sm"].tile([1, T], F32, name=f"wexp{b}{uid}", tag="wexp")
        sume = P["sm"].tile([1, 1], F32, name=f"sume{b}{uid}", tag="sume")
        nc.scalar.activation(wexp, ps, AF.Exp, bias=nm64, scale=1.0 / 64.0,
                             accum_out=sume)
        rsum = P["sm"].tile([1, 1], F32, name=f"rsum{b}{uid}", tag="rsum")
        nc.vector.reciprocal(rsum, sume)
        wnorm = P["sm"].tile([1, T], BF16, name=f"wnorm{b}{uid}", tag="wnorm")
        nc.vector.tensor_scalar_mul(wnorm, wexp, rsum)

        # broadcast weights to 128 partitions via DRAM round-trip
        nc.sync.dma_start(out=wscr[b:b + 1, :], in_=wnorm)
        wrep = P["wrep"].tile([128, T], BF16, name=f"wrep{b}{uid}", tag="wrep")
        nc.sync.dma_start(out=wrep, in_=_bcast(wscr[b:b + 1, :]))

        # appliedT[:, b] = sum_t enc * w  (bf16 path for accuracy)
        for k in range(KO):
            scr = P["scr"].tile([128, T], BF16, name=f"scr{b}_{k}{uid}",
                                tag="scr")
            nc.vector.scalar_tensor_tensor(
                out=scr, in0=encb[b][:, k, :], scalar=1.0, in1=wrep,
                op0=ALU.mult, op1=ALU.mult,
                accum_out=appT[k][:, b:b + 1],
            )

    # ---- main loop: flat pair slots; each score matmul is emitted one slot
    # late (crossing b boundaries) so the in-order PE never waits on tanh ----
    ps_tiles = {}
    pending = None            # (b, jp, ps, pair)
    for b in range(BL):
        ps_tiles[b] = P["ps"].tile([1, T], F32, name=f"ps{b}{uid}", tag="ps")
        for jp in range(JP):
            pair = P["act"].tile([128, 2, T], F8, name=f"act{b}_{jp}{uid}",
                                 tag="act")
            for jj in range(2):
                j = 2 * jp + jj
                pe = P["pe"].tile([128, T], F32, name=f"pe{b}_{j}{uid}", tag="pe")
                for kp in range(KP):
                    nc.tensor.matmul(
                        pe,
                        waET[:, 2 * kp:2 * kp + 2, j * 128:(j + 1) * 128],
                        enc[b][:, 2 * kp:2 * kp + 2, :],
                        start=(kp == 0), stop=(kp == KP - 1),
                        perf_mode=PM,
                    )
                nc.scalar.activation(pair[:, jj, :], pe, AF.Tanh,
                                     bias=hpT[:, j, b:b + 1])
            if pending is not None:
                pb, pjp, pps, ppair = pending
                nc.tensor.matmul(
                    pps, w2s[:, 2 * pjp:2 * pjp + 2, 0:1], ppair,
                    start=(pjp == 0), stop=(pjp == JP - 1), perf_mode=PM)
                if pjp == JP - 1:
                    _tail_b(pb, pps)
            pending = (b, jp, ps_tiles[b], pair)
    pb, pjp, pps, ppair = pending
    nc.tensor.matmul(pps, w2s[:, 2 * pjp:2 * pjp + 2, 0:1], ppair,
                     start=(pjp == 0), stop=(pjp == JP - 1), perf_mode=PM)
    _tail_b(pb, pps)

    # ---- epilogue ----
    appbf = []
    for k in range(KO):
        nc.sync.dma_start(out=appT_d[k], in_=appT[k])
        t_c = P["work"].tile([128, BL], F8, name=f"appbf{k}{uid}",
                             tag=f"appbf{k}")
        nc.vector.tensor_scalar_mul(t_c, appT[k], 16.0)
        appbf.append(t_c)

    out_sb = P["work"].tile([BL, D], F32, name=f"out_sb{uid}", tag="out_sb")
    for h in range(D // 512):
        pc = P["pc"].tile([BL, 512], F32, name=f"pc{h}{uid}", tag="pc")
        for k in range(KO):
            nc.tensor.matmul(
                pc, appbf[k], wces[:, k, h * 512:(h + 1) * 512],
                start=(k == 0), stop=(k == KO - 1),
            )
        psb = P["work"].tile([BL, 512], F32, name=f"psb{h}{uid}", tag="psb")
        nc.vector.scalar_tensor_tensor(
            out=psb, in0=pc, scalar=1.0 / 256.0,
            in1=dec[:, h * 512:(h + 1) * 512],
            op0=ALU.mult, op1=ALU.add)
        nc.scalar.activation(out_sb[:, h * 512:(h + 1) * 512], psb, AF.Tanh)
    nc.sync.dma_start(out=out_d, in_=out_sb)


def build_nc(reps=1):
    nc = bacc.Bacc("TRN2", target_bir_lowering=False, debug=False)
    ins = {}

    def din(name, shape, dt):
        ins[name] = nc.dram_tensor(name, shape, dt, kind="ExternalInput").ap()

    din("enc8", [BL, 128, KO, T], F8)
    din("encb", [BL, 128, KO, T], BF16)
    din("waET8", [128, KO, F], F8)
    din("w2s", [128, FJ, 16], F8)
    din("hpT", [128, FJ, BL], F32)
    din("wces8", [128, KO, D], F8)
    din("dec", [BL, D], F32)
    wscr = nc.dram_tensor("wscr", [BL, T], BF16, kind="Internal").ap()
    out_d = nc.dram_tensor("out", [BL, D], F32, kind="ExternalOutput").ap()
    appT_d = nc.dram_tensor("appliedT", [KO, 128, BL], F32,
                            kind="ExternalOutput").ap()
    with tile.TileContext(nc) as tc:
        with ExitStack() as ctx:
            P = {}

            def pool(key, bufs, space="SBUF"):
                P[key] = ctx.enter_context(
                    tc.tile_pool(name=f"p_{key}", bufs=bufs, space=space))

            pool("waET", 2)
            pool("enc", 1)
            pool("encb", 1)
            pool("w", 2)
            pool("tailc", 2)
            pool("work", 2)
            pool("act", 3)
            pool("wrep", 2)
            pool("scr", 2)
            pool("sm", 2)
            pool("pe", 5, "PSUM")
            pool("ps", 2, "PSUM")
            pool("pc", 1, "PSUM")
            for r in range(reps):
                _rep(tc, P, ins, wscr, out_d, appT_d, uid=f"r{r}")
    nc.compile()
    return nc


def _prep_inputs(hidden, decoder_out, encoder_states, Wa, ba, w2, Wc, bc):
    f8 = ml_dtypes.float8_e4m3
    bf = ml_dtypes.bfloat16
    f32 = np.float32

    hidden = np.asarray(hidden, f32)
    decoder_out = np.asarray(decoder_out, f32)
    Wa = np.asarray(Wa, f32)
    ba = np.asarray(ba, f32)
    w2 = np.asarray(w2, f32)
    Wc = np.asarray(Wc, f32)
    bc = np.asarray(bc, f32)

    # host-folded small projections
    h_proj = hidden @ Wa[:, :D].T + ba                      # [B, F] fp32
    dec_full = (decoder_out @ Wc[:, :D].T + bc).astype(f32)  # [B, D]

    WaE = Wa[:, D:]                                         # [F, E]
    waET8 = np.ascontiguousarray(
        WaE.T.reshape(KO, 128, F).transpose(1, 0, 2)).astype(f8)
    wces8 = np.ascontiguousarray(
        (Wc[:, D:] * 16.0).T.reshape(KO, 128, D).transpose(1, 0, 2)).astype(f8)
    w2s = np.zeros((128, FJ, 16), f32)
    w2s[:, :, 0] = (w2[0].reshape(FJ, 128) * 64.0).T
    w2s = w2s.astype(f8)

    enc_f32 = np.asarray(encoder_states, f32)               # [T, B, E]
    enc8_full = enc_f32.astype(f8)
    encb_full = enc_f32.astype(bf)

    shared = {"waET8": waET8, "w2s": w2s, "wces8": wces8}
    in_maps = []
    for c in range(NCORES):
        sl = slice(c * BL, (c + 1) * BL)
        m = dict(shared)
        m["enc8"] = np.ascontiguousarray(
            enc8_full[:, sl, :].reshape(T, BL, KO, 128).transpose(1, 3, 2, 0))
        m["encb"] = np.ascontiguousarray(
            encb_full[:, sl, :].reshape(T, BL, KO, 128).transpose(1, 3, 2, 0))
        m["hpT"] = np.ascontiguousarray(
            h_proj[sl].T.reshape(FJ, 128, BL).transpose(1, 0, 2)).astype(f32)
        m["dec"] = np.ascontiguousarray(dec_full[sl])
        in_maps.append(m)
    return in_maps


def kernel(hidden, decoder_out, encoder_states, Wa, ba, w2, b2, Wc, bc):
    global _nc_cache
    if _nc_cache is None:
        _nc_cache = build_nc()
    in_maps = _prep_inputs(hidden, decoder_out, encoder_states, Wa, ba, w2, Wc, bc)
    res = run_bass_kernel_spmd(_nc_cache, in_maps, core_ids=list(range(NCORES)))
    out = np.concatenate([res.results[c]["out"] for c in range(NCORES)], axis=0)
    applied = np.concatenate(
        [res.results[c]["appliedT"].transpose(2, 0, 1).reshape(BL, E)
         for c in range(NCORES)], axis=0)
    return out.astype(np.float32), applied.astype(np.float32)



# revision 81
# speedup vs baseline: 1.1777x; 1.1777x over previous
"""Trainium2 Bass/Tile kernel for the AttentionModule problem — v3.

Computation (per the reference):
    h_proj  = hidden @ Wa[:, :D].T + ba                       [B, 2E]   (host)
    e_proj  = einsum('tbe,fe->tbf', enc, Wa[:, D:])           [T, B, 2E] (PE, fp8 DoubleRow)
    act     = tanh(h_proj + e_proj)                           (ACT, bias=hpT)
    scores  = einsum('tbf,f->bt', act, w2[0])                 (PE, fp8 DoubleRow)
    weights = softmax(scores, axis=t)                         (no max-subtraction:
                                                               |scores| is O(1))
    applied = einsum('bt,tbe->be', weights, enc)              (DVE stt, bf16)
    out     = tanh(cat(decoder_out, applied) @ Wc.T + bc)     (decoder half on host;
                                                               applied half on PE fp8,
                                                               transposed [d,b] output)

Cost-model-driven design (TimelineSim is the graded clock):
  * ACT is the bottleneck engine (~83us busy: 128 tanh instrs + 5 exp).
    Everything else (PE ~65us, DVE ~50us, DMA ~45us, Pool ~5us) is scheduled
    to keep the ACT queue never-stalling: score matmuls are emitted LAG pair
    slots late so exp only waits ~300ns behind ACT's own stream.
  * exp is batched across b-pairs (b0..b5) in [2,T] psum tiles - one exp
    instruction covers two rows at the same 944ns cost. b6/b7 stay solo so
    the tail depends only on b7's chain.
  * Softmax skips the max-subtraction (scores are bounded by |w2|_1 ~ O(1)).
  * weights broadcast [1,T]->[128,T] via a PE ones-matmul into PSUM; the
    applied stt ops read that PSUM tile directly (no copy).
  * Head: DMAs split across the SP and DVE HWDGE queues (issue rate is
    650ns/DMA per queue) and enc8[0] split in k-halves so the first tanh
    fires at ~5us; a dummy activation at t~0 forces the ACT table load early.
  * Tail: b7's applied stts split DVE(k0-3)/Pool(k4-7) in parallel, combine
    matmul pairs emitted in completion order, epilogue split in ch-halves
    (Pool stt + ACT tanh + DMA overlap).
  * fp8 DoubleRow for e_proj/scores/combine (0.5 cyc/row in the cost model).

Scaling ladder (fp8e4m3 normals start at 2^-6, small weights pre-scaled):
  w2 x64 on host -> scores psum = 64*s -> exp(scale=1/64).
  wnorm = wexp * rsum * 16 -> appT = 16*applied (host divides appliedT by 16).
  WcE x16 on host -> combine psum = 256*applied@WcE.T -> stt scale 1/256.
"""

import numpy as np
import ml_dtypes
from contextlib import ExitStack

import concourse.bass as bass
import concourse.tile as tile
from concourse import bacc, mybir
from concourse.bass_utils import run_bass_kernel_spmd

B, T, E, D = 64, 512, 1024, 1024
NCORES = 8
BL = B // NCORES          # 8 batch rows per core
F = 2 * E                 # 2048
KO = E // 128             # 8 contraction sub-tiles
KP = KO // 2              # 4 DoubleRow k-pairs
FJ = F // 128             # 16 f-tiles
JP = FJ // 2              # 8 f-tile pairs
CH = D // 128             # 8 output d-chunks
F8 = mybir.dt.float8e4
BF16 = mybir.dt.bfloat16
F32 = mybir.dt.float32
AF = mybir.ActivationFunctionType
ALU = mybir.AluOpType
PM = mybir.MatmulPerfMode.DoubleRow

LAG = 2                   # pair-slots of lateness for score matmuls

# tanh(x) ~= x*(PC0 + PC1 x^2 + PC2 x^4 + PC3 x^6) on [-PCLAMP, PCLAMP],
# gaussian(0.6)-weighted fit: rms err ~4e-3 under the actual hp+e_proj
# distribution (sigma ~0.58) - small next to the existing fp8 act
# quantization (rms 1.2e-2).  Used for tiles offloaded from ACT to DVE.
PCLAMP = 3.0
PC0, PC1, PC2, PC3 = 0.99296556, -0.28696134, 0.05910922, -0.00418261
# (b, jp) pairs computed on DVE instead of ACT.  Whole pairs only: the two
# halves share one fp8 tile, and cross-engine writes to a tile serialize.
# Their score matmul is reordered to the end of the row's accumulation so
# the in-order PE queue never waits on the slower poly.
OFFLOAD = {(0, 3), (0, 4), (1, 0), (3, 0)}

_nc_cache = None


def _rep(tc, P, ins, out_d, appT_d, uid=""):
    nc = tc.nc

    # ---- warm up the ACT table (Tanh/Exp set) as early as possible ----
    ones = P["c"].tile([33, 128], BF16, name=f"ones{uid}", tag="ones")
    nc.gpsimd.memset(ones[:], 1.0)
    warm = P["c"].tile([1, 1], BF16, name=f"warm{uid}", tag="warm")
    nc.scalar.activation(warm, ones[0:1, 0:1], AF.Tanh, bias=ones[0:1, 0:1])

    # ---- input loads, all on the SP HWDGE queue (650ns/issue) -----------
    # Arrival order is tuned against need times: hpT+waJ0+enc8[0] feed the
    # first tanh at ~5.7us; waJ groups grow as PE's j-pace allows; enc8[b]
    # stays ahead of PE's 7.9us/row pace; encb[b] ahead of the softmax tails.
    hpT = P["c"].tile([128, FJ, BL], F32, name=f"hpT{uid}", tag="hpT")
    nc.sync.dma_start(out=hpT, in_=ins["hpT"])
    enc = [None] * BL
    encb = [None] * BL
    # waJ grouped tiles: [j0], [j1], [j2-3], [j4-7], [j8-11], [j12-15]
    wgroups = [(0, 1), (1, 2), (2, 4), (4, 8), (8, 12), (12, 16)]
    wtiles = {}

    def load_wgroup(gi, eng=None):
        lo, hi = wgroups[gi]
        t_w = P["wa"].tile([128, hi - lo, KO, 128], F8, name=f"waG{gi}{uid}",
                           tag=f"waG{gi}")
        (eng or nc.sync).dma_start(out=t_w, in_=ins["waJ"][:, lo:hi])
        for j in range(lo, hi):
            wtiles[j] = (t_w, j - lo)

    # j0/j1 issue via the Pool SWDGE (separate descriptor generator from the
    # shared HWDGE unit) so their transfers interleave with SP's enc8[0]
    # halves from t~2us.
    load_wgroup(0, nc.gpsimd)
    load_wgroup(1, nc.gpsimd)
    enc[0] = P["enc"].tile([128, KO, T], F8, name=f"enc0{uid}", tag="enc0")
    nc.sync.dma_start(out=enc[0][:, 0:KO // 2, :],
                      in_=ins["enc8"][0, :, 0:KO // 2, :])
    nc.sync.dma_start(out=enc[0][:, KO // 2:, :],
                      in_=ins["enc8"][0, :, KO // 2:, :])
    w2s = P["c"].tile([128, FJ, 16], F8, name=f"w2s{uid}", tag="w2s")
    nc.sync.dma_start(out=w2s, in_=ins["w2s"])
    for gi in range(2, len(wgroups)):
        load_wgroup(gi)
    # Every encb row is swept by DVE (k0-3) and Pool (k4-7) concurrently;
    # accesses to a single tile serialize across engines, so load each row
    # as two half tiles.
    encbH = {}

    def load_encb(b):
        for h in range(2):
            t_e = P["encb"].tile([128, KO // 2, T], BF16,
                                 name=f"encb{b}_{h}{uid}", tag=f"encb{b}_{h}")
            nc.sync.dma_start(
                out=t_e,
                in_=ins["encb"][b, :, h * (KO // 2):(h + 1) * (KO // 2), :])
            encbH[(b, h)] = t_e

    for b in range(1, BL):
        enc[b] = P["enc"].tile([128, KO, T], F8, name=f"enc{b}{uid}",
                               tag=f"enc{b}")
        nc.sync.dma_start(out=enc[b], in_=ins["enc8"][b])
        load_encb(b - 1)
    load_encb(BL - 1)
    wces = P["c"].tile([128, CH, KO, 128], F8, name=f"wces{uid}", tag="wces")
    nc.sync.dma_start(out=wces, in_=ins["wcesT"])
    decT = P["c"].tile([128, CH, BL], F32, name=f"decT{uid}", tag="decT")
    nc.sync.dma_start(out=decT, in_=ins["decT"])

    # applied^T accumulators (fp32, each column written once), one tile per
    # DoubleRow pair so converts/stores wait only on their own pair; in the
    # b7 tail DVE sweeps kp0/kp1 while Pool sweeps kp2/kp3 in parallel
    appF = [P["c"].tile([128, 2, BL], F32, name=f"appF{kp}{uid}",
                        tag=f"appF{kp}") for kp in range(KP)]

    def appT(k):
        return appF[k // 2][:, k % 2, :]

    # fp8 copies, paired for DoubleRow combine: [kp][128, 2, BL]
    appbf = [P["c"].tile([128, 2, BL], F8, name=f"appbf{kp}{uid}",
                         tag=f"appbf{kp}") for kp in range(KP)]
    outP = P["pc"].tile([128, CH, BL], F32, name=f"outP{uid}", tag="outP")
    # zero once and accumulate with start=False throughout: a start=True
    # matmul wipes the other chunks' partial sums sharing this psum tile
    nc.vector.memset(outP[:], 0.0)

    # scores psum tiles, one [1, T] per row.  Rows (0,1),(2,3),(4,5) pair up
    # for the softmax head: DVE copies each row's scores into halves of one
    # [1, 2T] SBUF tile so a single exp instruction covers both rows (saves
    # ~0.9us of ACT per pair); b6/b7 stay solo to keep the tail short.
    ps2 = {}
    sexp = {}

    def ps_tile(b):
        if b not in ps2:
            ps2[b] = P["ps"].tile([1, T], F32, name=f"ps{b}{uid}", tag="ps")
        return ps2[b], 0

    def stash_scores(b):
        """copy row b's scores psum into its half of the pair's SBUF tile"""
        g = b // 2
        if g not in sexp:
            sexp[g] = P["sm"].tile([1, 2 * T], F32, name=f"sexp{g}{uid}",
                                   tag="sexp")
        i = b % 2
        nc.vector.tensor_copy(sexp[g][:, i * T:(i + 1) * T], ps2[b])

    def softmax_head(rows):
        """exp + normalization for a stashed pair or a solo psum row."""
        n = len(rows)
        b0 = rows[0]
        g = b0 // 2
        if n == 2:
            wexp = P["sm"].tile([1, 2 * T], BF16, name=f"wexp{g}{uid}",
                                tag="wexp")
            nc.scalar.activation(wexp, sexp[g], AF.Exp, scale=1.0 / 64.0)
            sums = P["sm"].tile([1, 2], F32, name=f"sums{g}{uid}", tag="sums")
            junk = P["sm"].tile([1, T], BF16, name=f"junk{g}{uid}", tag="junk")
            for i in range(2):
                nc.vector.tensor_scalar(
                    out=junk, in0=wexp[:, i * T:(i + 1) * T], scalar1=1.0,
                    scalar2=0.0, op0=ALU.mult, op1=ALU.add,
                    accum_out=sums[:, i:i + 1])
            rs = P["sm"].tile([1, 2], F32, name=f"rs{g}{uid}", tag="rs")
            nc.vector.reciprocal(rs, sums)
            srcs = [(wexp[:, i * T:(i + 1) * T], rs[:, i:i + 1])
                    for i in range(2)]
        else:
            wexp = P["sm"].tile([1, T], BF16, name=f"wexpS{b0}{uid}",
                                tag="wexpS")
            sume = P["sm"].tile([1, 1], F32, name=f"sumeS{b0}{uid}",
                                tag="sumeS")
            nc.scalar.activation(wexp, ps2[b0], AF.Exp,
                                 scale=1.0 / 64.0, accum_out=sume)
            rs = P["sm"].tile([1, 1], F32, name=f"rsS{b0}{uid}", tag="rsS")
            nc.vector.reciprocal(rs, sume)
            srcs = [(wexp[:], rs[:])]
        out = []
        for (src, rsv), b in zip(srcs, rows):
            wn = P["sm"].tile([1, T], BF16, name=f"wn{b}{uid}",
                              tag=f"wn{b % 2}")
            nc.vector.tensor_scalar(out=wn, in0=src,
                                    scalar1=rsv, scalar2=16.0,
                                    op0=ALU.mult, op1=ALU.mult)
            out.append((b, wn[0:1, :]))
        return out

    def bcast(b, wnorm_row):
        """weights [1,T] -> [128,T] bf16 SBUF via the Pool engine's native
        partition_broadcast (GPSIMD cannot touch PSUM; a PE ones-matmul
        would strand the result there)."""
        wrep = P["wrp"].tile([128, T], BF16, name=f"wrep{b}{uid}", tag="wrep")
        nc.gpsimd.partition_broadcast(wrep, wnorm_row)
        return wrep

    def applied_k(b, k, wsrc, engine, pool):
        scr = P[pool].tile([128, T], BF16, name=f"scr{b}_{k}{uid}", tag=pool)
        src = encbH[(b, k // (KO // 2))][:, k % (KO // 2), :]
        engine.scalar_tensor_tensor(
            out=scr, in0=src, scalar=1.0, in1=wsrc,
            op0=ALU.mult, op1=ALU.mult,
            accum_out=appT(k)[:, b:b + 1])

    def combine_kp(kp, first, last):
        """fp8-convert appT pair kp, store its appliedT slice, and run its
        chunk-matmuls.  Plain fp8 (8-row outputs are nearly free; DoubleRow
        mis-pairs the 8-byte-stride moving operand)."""
        nc.vector.tensor_scalar_mul(appbf[kp], appF[kp], 1.0)
        nc.sync.dma_start(out=appT_d[:, kp], in_=appF[kp])
        for ch in range(CH):
            for kk in range(2):
                nc.tensor.matmul(
                    outP[:, ch, :], wces[:, ch, 2 * kp + kk, :],
                    appbf[kp][:, kk, :], start=False,
                    stop=(last and kk == 1), skip_group_check=True)

    def poly_tanh(out_f8, pe, hp_ap, nm):
        """tanh via clamped odd polynomial on the DVE (offloads the ACT
        bottleneck).  ~2.8us of DVE vs 0.61us of ACT per tile."""
        xt = P["px"].tile([128, T], BF16, name=f"x{nm}{uid}", tag="pxX")
        nc.vector.tensor_scalar(out=xt, in0=pe, scalar1=hp_ap,
                                scalar2=PCLAMP, op0=ALU.add, op1=ALU.min)
        x2 = P["px"].tile([128, T], BF16, name=f"x2{nm}{uid}", tag="pxX2")
        nc.vector.tensor_scalar(out=x2, in0=xt, scalar1=-PCLAMP,
                                scalar2=None, op0=ALU.max)
        t2 = P["px"].tile([128, T], BF16, name=f"t2{nm}{uid}", tag="pxT")
        nc.vector.tensor_tensor(out=t2, in0=x2, in1=x2, op=ALU.mult)
        u1 = P["px"].tile([128, T], BF16, name=f"u1{nm}{uid}", tag="pxU1")
        nc.vector.tensor_scalar(out=u1, in0=t2, scalar1=PC3,
                                scalar2=PC2, op0=ALU.mult, op1=ALU.add)
        u2 = P["px"].tile([128, T], BF16, name=f"u2{nm}{uid}", tag="pxU2")
        nc.vector.tensor_tensor(out=u2, in0=u1, in1=t2, op=ALU.mult)
        u3 = P["px"].tile([128, T], BF16, name=f"u3{nm}{uid}", tag="pxU1")
        nc.vector.tensor_scalar(out=u3, in0=u2, scalar1=PC1,
                                scalar2=None, op0=ALU.add)
        u4 = P["px"].tile([128, T], BF16, name=f"u4{nm}{uid}", tag="pxU2")
        nc.vector.tensor_tensor(out=u4, in0=u3, in1=t2, op=ALU.mult)
        u5 = P["px"].tile([128, T], BF16, name=f"u5{nm}{uid}", tag="pxU1")
        nc.vector.tensor_scalar(out=u5, in0=u4, scalar1=PC0,
                                scalar2=None, op0=ALU.add)
        nc.vector.tensor_tensor(out=out_f8, in0=u5, in1=x2, op=ALU.mult)

    # ---- deferred-emission machinery -------------------------------------
    pend = []                  # deferred (b, jp, pair) score matmuls
    tail_q = []                # (b, wnorm_row) rows ready for bcast+applied
    started = set()            # rows whose scores accumulation has begun
    held = {}                  # b -> [(jp, pair)] poly pairs, scored last

    def score_mm(b, jp, pair, stop):
        ps, row = ps_tile(b)
        nc.tensor.matmul(ps[row:row + 1, :], w2s[:, 2 * jp:2 * jp + 2, 0:1],
                         pair, start=(b not in started), stop=stop,
                         perf_mode=PM, skip_group_check=True)
        started.add(b)

    def finish_row(b):
        hl = held.pop(b, [])
        for i, (jp, pair) in enumerate(hl):
            score_mm(b, jp, pair, stop=(i == len(hl) - 1))
        if b < 6:
            stash_scores(b)
            if b % 2 == 1:
                tail_q.extend(softmax_head([b - 1, b]))
        else:
            rows = softmax_head([b])
            if b == BL - 1 and rest:
                rb, rwrep = rest.pop()
                for k in range(3, KO):
                    applied_k(rb, k, rwrep, nc.vector, "scrD")
            tail_q.extend(rows)

    def flush_one():
        b, jp, pair = pend.pop(0)
        score_mm(b, jp, pair, stop=(jp == JP - 1 and not held.get(b)))
        if jp == JP - 1:
            finish_row(b)

    rest = []                  # b6's deferred sweep continuation

    def emit_tail(b, wnorm_row):
        # The applied sweep is DVE-only (GPSIMD supports neither PSUM access
        # nor scalar_tensor_tensor); Pool's contribution is the broadcast.
        wrep = bcast(b, wnorm_row)
        if b == BL - 2:
            # emit only the first stts now; the rest go out after b7's
            # softmax ops so those don't queue behind the whole sweep
            for k in range(3):
                applied_k(b, k, wrep, nc.vector, "scrD")
            rest.append((b, wrep))
            return
        for k in range(KO):
            applied_k(b, k, wrep, nc.vector, "scrD")
            if b == BL - 1 and k in (2, 4, 6, 7):
                combine_kp({2: 0, 4: 1, 6: 2, 7: 3}[k],
                           k == 2, k == 7)

    # ---- main loop -------------------------------------------------------
    for b in range(BL):
        for jp in range(JP):
            # offloaded pairs live in their own pool: their score matmul is
            # deferred to the row end, which would otherwise WAR-block the
            # regular act-tile rotation
            off = (b, jp) in OFFLOAD
            pair = P["actP" if off else "act"].tile(
                [128, 2, T], F8, name=f"act{b}_{jp}{uid}",
                tag="actP" if off else "act")
            for jj in range(2):
                j = 2 * jp + jj
                wt, wi = wtiles[j]
                pe = P["pe"].tile([128, T], F32, name=f"pe{b}_{j}{uid}",
                                  tag="pe")
                for kp in range(KP):
                    nc.tensor.matmul(
                        pe,
                        wt[:, wi, 2 * kp:2 * kp + 2, :],
                        enc[b][:, 2 * kp:2 * kp + 2, :],
                        start=(kp == 0), stop=(kp == KP - 1),
                        perf_mode=PM)
                if (b, jp) in OFFLOAD:
                    poly_tanh(pair[:, jj, :], pe, hpT[:, j, b:b + 1],
                              f"{b}_{j}")
                else:
                    nc.scalar.activation(pair[:, jj, :], pe, AF.Tanh,
                                         bias=hpT[:, j, b:b + 1])
            if (b, jp) in OFFLOAD:
                held.setdefault(b, []).append((jp, pair))
            else:
                pend.append((b, jp, pair))
            if len(pend) > LAG:
                flush_one()
            if tail_q and len(pend) > LAG - 1:
                emit_tail(*tail_q.pop(0))
    while pend:
        flush_one()
    while tail_q:
        emit_tail(*tail_q.pop(0))

    # ---- epilogue: add decoder half, tanh, store (DVE: GPSIMD can't read
    # the PSUM accumulator) -----------------------------------------------
    pre = P["c"].tile([128, CH, BL], F32, name=f"pre{uid}", tag="pre")
    nc.vector.scalar_tensor_tensor(
        out=pre, in0=outP, scalar=1.0 / 256.0,
        in1=decT, op0=ALU.mult, op1=ALU.add)
    osb = P["c"].tile([128, CH, BL], F32, name=f"osb{uid}", tag="osb")
    nc.scalar.activation(osb, pre, AF.Tanh)
    nc.sync.dma_start(out=out_d, in_=osb)


def build_nc(reps=1):
    nc = bacc.Bacc("TRN2", target_bir_lowering=False, debug=False)
    ins = {}

    def din(name, shape, dt):
        ins[name] = nc.dram_tensor(name, shape, dt, kind="ExternalInput").ap()

    din("enc8", [BL, 128, KO, T], F8)
    din("encb", [BL, 128, KO, T], BF16)
    din("waJ", [128, FJ, KO, 128], F8)
    din("w2s", [128, FJ, 16], F8)
    din("hpT", [128, FJ, BL], F32)
    din("wcesT", [128, CH, KO, 128], F8)
    din("decT", [128, CH, BL], F32)
    out_d = nc.dram_tensor("outT", [128, CH, BL], F32,
                           kind="ExternalOutput").ap()
    appT_d = nc.dram_tensor("appliedT", [128, KP, 2, BL], F32,
                            kind="ExternalOutput").ap()
    with tile.TileContext(nc) as tc:
        with ExitStack() as ctx:
            P = {}

            def pool(key, bufs, space="SBUF"):
                P[key] = ctx.enter_context(
                    tc.tile_pool(name=f"p_{key}", bufs=bufs, space=space))

            pool("c", 2)        # constants / singletons
            pool("wa", 1)
            pool("enc", 1)
            pool("encb", 1)
            pool("act", 8)
            pool("actP", 2)
            pool("scrD", 8)
            pool("sm", 3)
            pool("wrp", 3)
            pool("px", 2)
            pool("pe", 5, "PSUM")
            pool("ps", 2, "PSUM")
            pool("pc", 1, "PSUM")
            for r in range(reps):
                _rep(tc, P, ins, out_d, appT_d, uid=f"r{r}")
    nc.compile()
    return nc


def _prep_inputs(hidden, decoder_out, encoder_states, Wa, ba, w2, Wc, bc):
    f8 = ml_dtypes.float8_e4m3
    bf = ml_dtypes.bfloat16
    f32 = np.float32

    hidden = np.asarray(hidden, f32)
    decoder_out = np.asarray(decoder_out, f32)
    Wa = np.asarray(Wa, f32)
    ba = np.asarray(ba, f32)
    w2 = np.asarray(w2, f32)
    Wc = np.asarray(Wc, f32)
    bc = np.asarray(bc, f32)

    # host-folded small projections
    h_proj = hidden @ Wa[:, :D].T + ba                       # [B, F] fp32
    dec_full = (decoder_out @ Wc[:, :D].T + bc).astype(f32)  # [B, D]

    WaE = Wa[:, D:]                                          # [F, E]
    # waJ[p, j, k, c] = WaE[j*128+c, k*128+p]
    waJ = np.ascontiguousarray(
        WaE.T.reshape(KO, 128, FJ, 128).transpose(1, 2, 0, 3)).astype(f8)
    # wcesT[p, ch, k, c] = 16*WcE[ch*128+c, k*128+p]
    wcesT = np.ascontiguousarray(
        (Wc[:, D:] * 16.0).T.reshape(KO, 128, CH, 128)
        .transpose(1, 2, 0, 3)).astype(f8)
    w2s = np.zeros((128, FJ, 16), np.float32)
    w2s[:, :, 0] = (w2[0].reshape(FJ, 128) * 64.0).T
    w2s = w2s.astype(f8)

    enc_f32 = np.asarray(encoder_states, f32)                # [T, B, E]
    enc8_full = enc_f32.astype(f8)
    encb_full = enc_f32.astype(bf)

    shared = {"waJ": waJ, "w2s": w2s, "wcesT": wcesT}
    in_maps = []
    for c in range(NCORES):
        sl = slice(c * BL, (c + 1) * BL)
        m = dict(shared)
        m["enc8"] = np.ascontiguousarray(
            enc8_full[:, sl, :].reshape(T, BL, KO, 128).transpose(1, 3, 2, 0))
        m["encb"] = np.ascontiguousarray(
            encb_full[:, sl, :].reshape(T, BL, KO, 128).transpose(1, 3, 2, 0))
        m["hpT"] = np.ascontiguousarray(
            h_proj[sl].T.reshape(FJ, 128, BL).transpose(1, 0, 2)).astype(f32)
        # decT[p, ch, b] = dec_full[b, ch*128+p]
        m["decT"] = np.ascontiguousarray(
            dec_full[sl].T.reshape(CH, 128, BL).transpose(1, 0, 2))
        in_maps.append(m)
    return in_maps


def kernel(hidden, decoder_out, encoder_states, Wa, ba, w2, b2, Wc, bc):
    global _nc_cache
    if _nc_cache is None:
        _nc_cache = build_nc()
    in_maps = _prep_inputs(hidden, decoder_out, encoder_states, Wa, ba, w2,
                           Wc, bc)
    res = run_bass_kernel_spmd(_nc_cache, in_maps, core_ids=list(range(NCORES)))
    # outT[p, ch, b] -> out[b, ch*128+p]
    out = np.concatenate(
        [res.results[c]["outT"].transpose(2, 1, 0).reshape(BL, D)
         for c in range(NCORES)], axis=0)
    applied = np.concatenate(
        [res.results[c]["appliedT"].reshape(128, KO, BL)
         .transpose(2, 1, 0).reshape(BL, E)
         for c in range(NCORES)], axis=0) * (1.0 / 16.0)
    return out.astype(np.float32), applied.astype(np.float32)


# revision 86
# speedup vs baseline: 1.3320x; 1.1310x over previous
"""Trainium2 Bass/Tile kernel for the AttentionModule problem — v3.

Computation (per the reference):
    h_proj  = hidden @ Wa[:, :D].T + ba                       [B, 2E]   (host)
    e_proj  = einsum('tbe,fe->tbf', enc, Wa[:, D:])           [T, B, 2E] (PE, fp8 DoubleRow)
    act     = tanh(h_proj + e_proj)                           (ACT, bias=hpT)
    scores  = einsum('tbf,f->bt', act, w2[0])                 (PE, fp8 DoubleRow)
    weights = softmax(scores, axis=t)                         (no max-subtraction:
                                                               |scores| is O(1))
    applied = einsum('bt,tbe->be', weights, enc)              (DVE stt, bf16)
    out     = tanh(cat(decoder_out, applied) @ Wc.T + bc)     (decoder half on host;
                                                               applied half on PE fp8,
                                                               transposed [d,b] output)

Cost-model-driven design (TimelineSim is the graded clock; ~97us standalone
vs the 124.7us baseline):
  * ACT is the bottleneck engine (~80us busy: tanh instrs + exp).  Everything
    else (DVE ~70us, PE ~64us, DMA ~45us) is scheduled to keep the ACT queue
    never-stalling: score matmuls are emitted LAG pair slots late so exp only
    waits ~300ns behind ACT's own stream.
  * Four tanh pairs (rows 0-3) are offloaded to the DVE as a clamped deg-7
    odd polynomial; their score matmuls are reordered to the end of the
    row's accumulation so the in-order PE queue never waits on the poly.
  * exp is batched across b-pairs (b0..b5): DVE copies both rows' score
    psums into one [1,2T] SBUF tile, a single exp covers the pair, and
    ts+accum ops recover the per-row sums.  b6/b7 stay solo (short tail).
  * Softmax skips the max-subtraction (scores are bounded by |w2|_1 ~ O(1)).
  * weights broadcast [1,T]->[128,T] via Pool's native partition_broadcast
    (GPSIMD can't touch PSUM, and can't run scalar_tensor_tensor at all);
    the applied sweep is DVE-only, reading SBUF at 594ns/stt.
  * Head: waJ j0/j1 issue via the Pool SWDGE (separate generator from the
    shared HWDGE), enc8[0] split in k-halves, a dummy activation forces the
    ACT table load early, and a PE dummy-matmul chain warms the p-state so
    row 0 runs at full clock.
  * Tail: b6's sweep is split around b7's softmax ops (head-blocking FIFO
    queues), converts/combine interleave per k-pair, outP is zeroed once
    and accumulated with start=False (a start=True matmul wipes the other
    chunks sharing the psum tile).
  * fp8 DoubleRow for e_proj/scores (0.5 cyc/row); combine runs transposed
    ([d,b] output, 8-row matmuls) in plain fp8.

Scaling ladder (fp8e4m3 normals start at 2^-6, small weights pre-scaled):
  w2 x64 on host -> scores psum = 64*s -> exp(scale=1/64).
  wnorm = wexp * rsum * 16 -> appT = 16*applied (host divides appliedT by 16).
  WcE x16 on host -> combine psum = 256*applied@WcE.T -> stt scale 1/256.
"""

import numpy as np
import ml_dtypes
from contextlib import ExitStack

import concourse.bass as bass
import concourse.tile as tile
from concourse import bacc, mybir
from concourse.bass_utils import run_bass_kernel_spmd

B, T, E, D = 64, 512, 1024, 1024
NCORES = 8
BL = B // NCORES          # 8 batch rows per core
F = 2 * E                 # 2048
KO = E // 128             # 8 contraction sub-tiles
KP = KO // 2              # 4 DoubleRow k-pairs
FJ = F // 128             # 16 f-tiles
JP = FJ // 2              # 8 f-tile pairs
CH = D // 128             # 8 output d-chunks
F8 = mybir.dt.float8e4
BF16 = mybir.dt.bfloat16
F32 = mybir.dt.float32
AF = mybir.ActivationFunctionType
ALU = mybir.AluOpType
PM = mybir.MatmulPerfMode.DoubleRow

LAG = 2                   # pair-slots of lateness for score matmuls

# tanh(x) ~= x*(PC0 + PC1 x^2 + PC2 x^4 + PC3 x^6) on [-PCLAMP, PCLAMP],
# gaussian(0.6)-weighted fit: rms err ~4e-3 under the actual hp+e_proj
# distribution (sigma ~0.58) - small next to the existing fp8 act
# quantization (rms 1.2e-2).  Used for tiles offloaded from ACT to DVE.
PCLAMP = 3.0
PC0, PC1, PC2, PC3 = 0.99296556, -0.28696134, 0.05910922, -0.00418261
# (b, jp) pairs computed on DVE instead of ACT.  Whole pairs only: the two
# halves share one fp8 tile, and cross-engine writes to a tile serialize.
# Their score matmul is reordered to the end of the row's accumulation so
# the in-order PE queue never waits on the slower poly.
OFFLOAD = {(0, 3), (0, 4), (1, 0), (3, 0)}

_nc_cache = None


def _rep(tc, P, ins, out_d, appT_d, uid=""):
    nc = tc.nc

    # ---- warm up the ACT table (Tanh/Exp set) as early as possible ----
    ones = P["c"].tile([33, 128], BF16, name=f"ones{uid}", tag="ones")
    nc.gpsimd.memset(ones[:], 1.0)
    warm = P["c"].tile([1, 1], BF16, name=f"warm{uid}", tag="warm")
    nc.scalar.activation(warm, ones[0:1, 0:1], AF.Tanh, bias=ones[0:1, 0:1])
    # PE p-state warmup: the tensor engine runs at half clock until it has
    # been continuously busy for 3us.  Dummy matmuls from t~1us get it to
    # full speed before the first e_proj tile, which would otherwise starve
    # ACT through row 0.
    wpe = P["pe"].tile([16, 128], F32, name=f"wpe{uid}", tag="pe")
    for i in range(28):
        nc.tensor.matmul(wpe, ones[0:1, 0:16], ones[0:1, :],
                         start=True, stop=True)

    # ---- input loads, all on the SP HWDGE queue (650ns/issue) -----------
    # Arrival order is tuned against need times: hpT+waJ0+enc8[0] feed the
    # first tanh at ~5.7us; waJ groups grow as PE's j-pace allows; enc8[b]
    # stays ahead of PE's 7.9us/row pace; encb[b] ahead of the softmax tails.
    hpT = P["c"].tile([128, FJ, BL], F32, name=f"hpT{uid}", tag="hpT")
    nc.sync.dma_start(out=hpT, in_=ins["hpT"])
    enc = [None] * BL
    encb = [None] * BL
    # waJ grouped tiles: [j0], [j1], [j2-3], [j4-7], [j8-11], [j12-15]
    wgroups = [(0, 1), (1, 2), (2, 4), (4, 8), (8, 12), (12, 16)]
    wtiles = {}

    def load_wgroup(gi, eng=None):
        lo, hi = wgroups[gi]
        t_w = P["wa"].tile([128, hi - lo, KO, 128], F8, name=f"waG{gi}{uid}",
                           tag=f"waG{gi}")
        (eng or nc.sync).dma_start(out=t_w, in_=ins["waJ"][:, lo:hi])
        for j in range(lo, hi):
            wtiles[j] = (t_w, j - lo)

    # j0/j1 issue via the Pool SWDGE (separate descriptor generator from the
    # shared HWDGE unit) so their transfers interleave with SP's enc8[0]
    # halves from t~2us.
    load_wgroup(0, nc.gpsimd)
    load_wgroup(1, nc.gpsimd)
    enc[0] = P["enc"].tile([128, KO, T], F8, name=f"enc0{uid}", tag="enc0")
    nc.sync.dma_start(out=enc[0][:, 0:KO // 2, :],
                      in_=ins["enc8"][0, :, 0:KO // 2, :])
    nc.sync.dma_start(out=enc[0][:, KO // 2:, :],
                      in_=ins["enc8"][0, :, KO // 2:, :])
    w2s = P["c"].tile([128, FJ, 16], F8, name=f"w2s{uid}", tag="w2s")
    nc.sync.dma_start(out=w2s, in_=ins["w2s"])
    for gi in range(2, len(wgroups)):
        load_wgroup(gi)
    # Every encb row is swept by DVE (k0-3) and Pool (k4-7) concurrently;
    # accesses to a single tile serialize across engines, so load each row
    # as two half tiles.
    encbH = {}

    def load_encb(b):
        for h in range(2):
            t_e = P["encb"].tile([128, KO // 2, T], BF16,
                                 name=f"encb{b}_{h}{uid}", tag=f"encb{b}_{h}")
            nc.sync.dma_start(
                out=t_e,
                in_=ins["encb"][b, :, h * (KO // 2):(h + 1) * (KO // 2), :])
            encbH[(b, h)] = t_e

    for b in range(1, BL):
        enc[b] = P["enc"].tile([128, KO, T], F8, name=f"enc{b}{uid}",
                               tag=f"enc{b}")
        nc.sync.dma_start(out=enc[b], in_=ins["enc8"][b])
        load_encb(b - 1)
    load_encb(BL - 1)
    wces = P["c"].tile([128, CH, KO, 128], F8, name=f"wces{uid}", tag="wces")
    nc.sync.dma_start(out=wces, in_=ins["wcesT"])
    decT = P["c"].tile([128, CH, BL], F32, name=f"decT{uid}", tag="decT")
    nc.sync.dma_start(out=decT, in_=ins["decT"])

    # applied^T accumulators (fp32, each column written once), one tile per
    # DoubleRow pair so converts/stores wait only on their own pair; in the
    # b7 tail DVE sweeps kp0/kp1 while Pool sweeps kp2/kp3 in parallel
    appF = [P["c"].tile([128, 2, BL], F32, name=f"appF{kp}{uid}",
                        tag=f"appF{kp}") for kp in range(KP)]

    def appT(k):
        return appF[k // 2][:, k % 2, :]

    # fp8 copies, paired for DoubleRow combine: [kp][128, 2, BL]
    appbf = [P["c"].tile([128, 2, BL], F8, name=f"appbf{kp}{uid}",
                         tag=f"appbf{kp}") for kp in range(KP)]
    outP = P["pc"].tile([128, CH, BL], F32, name=f"outP{uid}", tag="outP")
    # zero once and accumulate with start=False throughout: a start=True
    # matmul wipes the other chunks' partial sums sharing this psum tile
    nc.vector.memset(outP[:], 0.0)

    # scores psum tiles, one [1, T] per row.  Rows (0,1),(2,3),(4,5) pair up
    # for the softmax head: DVE copies each row's scores into halves of one
    # [1, 2T] SBUF tile so a single exp instruction covers both rows (saves
    # ~0.9us of ACT per pair); b6/b7 stay solo to keep the tail short.
    ps2 = {}
    sexp = {}

    def ps_tile(b):
        if b not in ps2:
            ps2[b] = P["ps"].tile([1, T], F32, name=f"ps{b}{uid}", tag="ps")
        return ps2[b], 0

    def stash_scores(b):
        """copy row b's scores psum into its half of the pair's SBUF tile"""
        g = b // 2
        if g not in sexp:
            sexp[g] = P["sm"].tile([1, 2 * T], F32, name=f"sexp{g}{uid}",
                                   tag="sexp")
        i = b % 2
        nc.vector.tensor_copy(sexp[g][:, i * T:(i + 1) * T], ps2[b])

    def softmax_head(rows):
        """exp + normalization for a stashed pair or a solo psum row."""
        n = len(rows)
        b0 = rows[0]
        g = b0 // 2
        if n == 2:
            wexp = P["sm"].tile([1, 2 * T], BF16, name=f"wexp{g}{uid}",
                                tag="wexp")
            nc.scalar.activation(wexp, sexp[g], AF.Exp, scale=1.0 / 64.0)
            sums = P["sm"].tile([1, 2], F32, name=f"sums{g}{uid}", tag="sums")
            junk = P["sm"].tile([1, T], BF16, name=f"junk{g}{uid}", tag="junk")
            for i in range(2):
                nc.vector.tensor_scalar(
                    out=junk, in0=wexp[:, i * T:(i + 1) * T], scalar1=1.0,
                    scalar2=0.0, op0=ALU.mult, op1=ALU.add,
                    accum_out=sums[:, i:i + 1])
            rs = P["sm"].tile([1, 2], F32, name=f"rs{g}{uid}", tag="rs")
            nc.vector.reciprocal(rs, sums)
            srcs = [(wexp[:, i * T:(i + 1) * T], rs[:, i:i + 1])
                    for i in range(2)]
        else:
            wexp = P["sm"].tile([1, T], BF16, name=f"wexpS{b0}{uid}",
                                tag="wexpS")
            sume = P["sm"].tile([1, 1], F32, name=f"sumeS{b0}{uid}",
                                tag="sumeS")
            nc.scalar.activation(wexp, ps2[b0], AF.Exp,
                                 scale=1.0 / 64.0, accum_out=sume)
            rs = P["sm"].tile([1, 1], F32, name=f"rsS{b0}{uid}", tag="rsS")
            nc.vector.reciprocal(rs, sume)
            srcs = [(wexp[:], rs[:])]
        out = []
        for (src, rsv), b in zip(srcs, rows):
            wn = P["sm"].tile([1, T], BF16, name=f"wn{b}{uid}",
                              tag=f"wn{b % 2}")
            nc.vector.tensor_scalar(out=wn, in0=src,
                                    scalar1=rsv, scalar2=16.0,
                                    op0=ALU.mult, op1=ALU.mult)
            out.append((b, wn[0:1, :]))
        return out

    def bcast(b, wnorm_row):
        """weights [1,T] -> [128,T] bf16 SBUF via the Pool engine's native
        partition_broadcast (GPSIMD cannot touch PSUM; a PE ones-matmul
        would strand the result there)."""
        wrep = P["wrp"].tile([128, T], BF16, name=f"wrep{b}{uid}", tag="wrep")
        nc.gpsimd.partition_broadcast(wrep, wnorm_row)
        return wrep

    def applied_k(b, k, wsrc, engine, pool):
        scr = P[pool].tile([128, T], BF16, name=f"scr{b}_{k}{uid}", tag=pool)
        src = encbH[(b, k // (KO // 2))][:, k % (KO // 2), :]
        engine.scalar_tensor_tensor(
            out=scr, in0=src, scalar=1.0, in1=wsrc,
            op0=ALU.mult, op1=ALU.mult,
            accum_out=appT(k)[:, b:b + 1])

    def combine_kp(kp, first, last):
        """fp8-convert appT pair kp, store its appliedT slice, and run its
        chunk-matmuls.  Plain fp8 (8-row outputs are nearly free; DoubleRow
        mis-pairs the 8-byte-stride moving operand)."""
        nc.vector.tensor_scalar_mul(appbf[kp], appF[kp], 1.0)
        nc.sync.dma_start(out=appT_d[:, kp], in_=appF[kp])
        for ch in range(CH):
            for kk in range(2):
                nc.tensor.matmul(
                    outP[:, ch, :], wces[:, ch, 2 * kp + kk, :],
                    appbf[kp][:, kk, :], start=False,
                    stop=(last and kk == 1), skip_group_check=True)

    def poly_tanh(out_f8, pe, hp_ap, nm):
        """tanh via clamped odd polynomial on the DVE (offloads the ACT
        bottleneck).  ~2.8us of DVE vs 0.61us of ACT per tile."""
        xt = P["px"].tile([128, T], BF16, name=f"x{nm}{uid}", tag="pxX")
        nc.vector.tensor_scalar(out=xt, in0=pe, scalar1=hp_ap,
                                scalar2=PCLAMP, op0=ALU.add, op1=ALU.min)
        x2 = P["px"].tile([128, T], BF16, name=f"x2{nm}{uid}", tag="pxX2")
        nc.vector.tensor_scalar(out=x2, in0=xt, scalar1=-PCLAMP,
                                scalar2=None, op0=ALU.max)
        t2 = P["px"].tile([128, T], BF16, name=f"t2{nm}{uid}", tag="pxT")
        nc.vector.tensor_tensor(out=t2, in0=x2, in1=x2, op=ALU.mult)
        u1 = P["px"].tile([128, T], BF16, name=f"u1{nm}{uid}", tag="pxU1")
        nc.vector.tensor_scalar(out=u1, in0=t2, scalar1=PC3,
                                scalar2=PC2, op0=ALU.mult, op1=ALU.add)
        u2 = P["px"].tile([128, T], BF16, name=f"u2{nm}{uid}", tag="pxU2")
        nc.vector.tensor_tensor(out=u2, in0=u1, in1=t2, op=ALU.mult)
        u3 = P["px"].tile([128, T], BF16, name=f"u3{nm}{uid}", tag="pxU1")
        nc.vector.tensor_scalar(out=u3, in0=u2, scalar1=PC1,
                                scalar2=None, op0=ALU.add)
        u4 = P["px"].tile([128, T], BF16, name=f"u4{nm}{uid}", tag="pxU2")
        nc.vector.tensor_tensor(out=u4, in0=u3, in1=t2, op=ALU.mult)
        u5 = P["px"].tile([128, T], BF16, name=f"u5{nm}{uid}", tag="pxU1")
        nc.vector.tensor_scalar(out=u5, in0=u4, scalar1=PC0,
                                scalar2=None, op0=ALU.add)
        nc.vector.tensor_tensor(out=out_f8, in0=u5, in1=x2, op=ALU.mult)

    # ---- deferred-emission machinery -------------------------------------
    pend = []                  # deferred (b, jp, pair) score matmuls
    tail_q = []                # (b, wnorm_row) rows ready for bcast+applied
    started = set()            # rows whose scores accumulation has begun
    held = {}                  # b -> [(jp, pair)] poly pairs, scored last

    def score_mm(b, jp, pair, stop):
        ps, row = ps_tile(b)
        nc.tensor.matmul(ps[row:row + 1, :], w2s[:, 2 * jp:2 * jp + 2, 0:1],
                         pair, start=(b not in started), stop=stop,
                         perf_mode=PM, skip_group_check=True)
        started.add(b)

    def finish_row(b):
        hl = held.pop(b, [])
        for i, (jp, pair) in enumerate(hl):
            score_mm(b, jp, pair, stop=(i == len(hl) - 1))
        if b < 6:
            stash_scores(b)
            if b % 2 == 1:
                tail_q.extend(softmax_head([b - 1, b]))
        else:
            rows = softmax_head([b])
            if b == BL - 1 and rest:
                rb, rwrep = rest.pop()
                for k in range(3, KO):
                    applied_k(rb, k, rwrep, nc.vector, "scrD")
            tail_q.extend(rows)

    def flush_one():
        b, jp, pair = pend.pop(0)
        score_mm(b, jp, pair, stop=(jp == JP - 1 and not held.get(b)))
        if jp == JP - 1:
            finish_row(b)

    rest = []                  # b6's deferred sweep continuation

    def emit_tail(b, wnorm_row):
        # The applied sweep is DVE-only (GPSIMD supports neither PSUM access
        # nor scalar_tensor_tensor); Pool's contribution is the broadcast.
        wrep = bcast(b, wnorm_row)
        if b == BL - 2:
            # emit only the first stts now; the rest go out after b7's
            # softmax ops so those don't queue behind the whole sweep
            for k in range(3):
                applied_k(b, k, wrep, nc.vector, "scrD")
            rest.append((b, wrep))
            return
        for k in range(KO):
            applied_k(b, k, wrep, nc.vector, "scrD")
            if b == BL - 1 and k in (2, 4, 6, 7):
                combine_kp({2: 0, 4: 1, 6: 2, 7: 3}[k],
                           k == 2, k == 7)

    # ---- main loop -------------------------------------------------------
    for b in range(BL):
        for jp in range(JP):
            # offloaded pairs live in their own pool: their score matmul is
            # deferred to the row end, which would otherwise WAR-block the
            # regular act-tile rotation
            off = (b, jp) in OFFLOAD
            pair = P["actP" if off else "act"].tile(
                [128, 2, T], F8, name=f"act{b}_{jp}{uid}",
                tag="actP" if off else "act")
            for jj in range(2):
                j = 2 * jp + jj
                wt, wi = wtiles[j]
                pe = P["pe"].tile([128, T], F32, name=f"pe{b}_{j}{uid}",
                                  tag="pe")
                for kp in range(KP):
                    nc.tensor.matmul(
                        pe,
                        wt[:, wi, 2 * kp:2 * kp + 2, :],
                        enc[b][:, 2 * kp:2 * kp + 2, :],
                        start=(kp == 0), stop=(kp == KP - 1),
                        perf_mode=PM)
                if (b, jp) in OFFLOAD:
                    poly_tanh(pair[:, jj, :], pe, hpT[:, j, b:b + 1],
                              f"{b}_{j}")
                else:
                    nc.scalar.activation(pair[:, jj, :], pe, AF.Tanh,
                                         bias=hpT[:, j, b:b + 1])
            if (b, jp) in OFFLOAD:
                held.setdefault(b, []).append((jp, pair))
            else:
                pend.append((b, jp, pair))
            if b >= 6 and jp == JP - 1:
                # tail rows: PE has no e_proj left to protect, so flush
                # immediately - exp lands right behind the row's last tanh
                # in the ACT queue, pulling the whole tail chain earlier
                while pend:
                    flush_one()
                while tail_q:
                    emit_tail(*tail_q.pop(0))
            if len(pend) > LAG:
                flush_one()
            if tail_q and len(pend) > LAG - 1:
                emit_tail(*tail_q.pop(0))
    while pend:
        flush_one()
    while tail_q:
        emit_tail(*tail_q.pop(0))

    # ---- epilogue: add decoder half, tanh, store (DVE: GPSIMD can't read
    # the PSUM accumulator) -----------------------------------------------
    pre = P["c"].tile([128, CH, BL], F32, name=f"pre{uid}", tag="pre")
    nc.vector.scalar_tensor_tensor(
        out=pre, in0=outP, scalar=1.0 / 256.0,
        in1=decT, op0=ALU.mult, op1=ALU.add)
    osb = P["c"].tile([128, CH, BL], F32, name=f"osb{uid}", tag="osb")
    nc.scalar.activation(osb, pre, AF.Tanh)
    nc.sync.dma_start(out=out_d, in_=osb)


def build_nc(reps=1):
    nc = bacc.Bacc("TRN2", target_bir_lowering=False, debug=False)
    ins = {}

    def din(name, shape, dt):
        ins[name] = nc.dram_tensor(name, shape, dt, kind="ExternalInput").ap()

    din("enc8", [BL, 128, KO, T], F8)
    din("encb", [BL, 128, KO, T], BF16)
    din("waJ", [128, FJ, KO, 128], F8)
    din("w2s", [128, FJ, 16], F8)
    din("hpT", [128, FJ, BL], F32)
    din("wcesT", [128, CH, KO, 128], F8)
    din("decT", [128, CH, BL], F32)
    out_d = nc.dram_tensor("outT", [128, CH, BL], F32,
                           kind="ExternalOutput").ap()
    appT_d = nc.dram_tensor("appliedT", [128, KP, 2, BL], F32,
                            kind="ExternalOutput").ap()
    with tile.TileContext(nc) as tc:
        with ExitStack() as ctx:
            P = {}

            def pool(key, bufs, space="SBUF"):
                P[key] = ctx.enter_context(
                    tc.tile_pool(name=f"p_{key}", bufs=bufs, space=space))

            pool("c", 2)        # constants / singletons
            pool("wa", 1)
            pool("enc", 1)
            pool("encb", 1)
            pool("act", 8)
            pool("actP", 2)
            pool("scrD", 8)
            pool("sm", 3)
            pool("wrp", 3)
            pool("px", 2)
            pool("pe", 5, "PSUM")
            pool("ps", 2, "PSUM")
            pool("pc", 1, "PSUM")
            for r in range(reps):
                _rep(tc, P, ins, out_d, appT_d, uid=f"r{r}")
    nc.compile()
    return nc


def _prep_inputs(hidden, decoder_out, encoder_states, Wa, ba, w2, Wc, bc):
    f8 = ml_dtypes.float8_e4m3
    bf = ml_dtypes.bfloat16
    f32 = np.float32

    hidden = np.asarray(hidden, f32)
    decoder_out = np.asarray(decoder_out, f32)
    Wa = np.asarray(Wa, f32)
    ba = np.asarray(ba, f32)
    w2 = np.asarray(w2, f32)
    Wc = np.asarray(Wc, f32)
    bc = np.asarray(bc, f32)

    # host-folded small projections
    h_proj = hidden @ Wa[:, :D].T + ba                       # [B, F] fp32
    dec_full = (decoder_out @ Wc[:, :D].T + bc).astype(f32)  # [B, D]

    WaE = Wa[:, D:]                                          # [F, E]
    # waJ[p, j, k, c] = WaE[j*128+c, k*128+p]
    waJ = np.ascontiguousarray(
        WaE.T.reshape(KO, 128, FJ, 128).transpose(1, 2, 0, 3)).astype(f8)
    # wcesT[p, ch, k, c] = 16*WcE[ch*128+c, k*128+p]
    wcesT = np.ascontiguousarray(
        (Wc[:, D:] * 16.0).T.reshape(KO, 128, CH, 128)
        .transpose(1, 2, 0, 3)).astype(f8)
    w2s = np.zeros((128, FJ, 16), np.float32)
    w2s[:, :, 0] = (w2[0].reshape(FJ, 128) * 64.0).T
    w2s = w2s.astype(f8)

    enc_f32 = np.asarray(encoder_states, f32)                # [T, B, E]
    enc8_full = enc_f32.astype(f8)
    encb_full = enc_f32.astype(bf)

    shared = {"waJ": waJ, "w2s": w2s, "wcesT": wcesT}
    in_maps = []
    for c in range(NCORES):
        sl = slice(c * BL, (c + 1) * BL)
        m = dict(shared)
        m["enc8"] = np.ascontiguousarray(
            enc8_full[:, sl, :].reshape(T, BL, KO, 128).transpose(1, 3, 2, 0))
        m["encb"] = np.ascontiguousarray(
            encb_full[:, sl, :].reshape(T, BL, KO, 128).transpose(1, 3, 2, 0))
        m["hpT"] = np.ascontiguousarray(
            h_proj[sl].T.reshape(FJ, 128, BL).transpose(1, 0, 2)).astype(f32)
        # decT[p, ch, b] = dec_full[b, ch*128+p]
        m["decT"] = np.ascontiguousarray(
            dec_full[sl].T.reshape(CH, 128, BL).transpose(1, 0, 2))
        in_maps.append(m)
    return in_maps


def kernel(hidden, decoder_out, encoder_states, Wa, ba, w2, b2, Wc, bc):
    global _nc_cache
    if _nc_cache is None:
        _nc_cache = build_nc()
    in_maps = _prep_inputs(hidden, decoder_out, encoder_states, Wa, ba, w2,
                           Wc, bc)
    res = run_bass_kernel_spmd(_nc_cache, in_maps, core_ids=list(range(NCORES)))
    # outT[p, ch, b] -> out[b, ch*128+p]
    out = np.concatenate(
        [res.results[c]["outT"].transpose(2, 1, 0).reshape(BL, D)
         for c in range(NCORES)], axis=0)
    applied = np.concatenate(
        [res.results[c]["appliedT"].reshape(128, KO, BL)
         .transpose(2, 1, 0).reshape(BL, E)
         for c in range(NCORES)], axis=0) * (1.0 / 16.0)
    return out.astype(np.float32), applied.astype(np.float32)
